# revision 71
# baseline (speedup 1.0000x reference)
"""Trainium2 Bass kernel for bipartite GNN metapath aggregation (LATTE).

Reference math:
    h_a = relu(x_a @ W_a + b_a); h_b = relu(x_b @ W_b + b_b)
    A[r,c] = #edges_ab(r,c); B[r,c] = #edges_ba(r,c)
    deg = colcount(A) + rowcount(B); d = 1/deg (0 where deg==0)
    out = (A*d) @ B @ h_a + A @ h_b
Reassociated (exact up to fp reassociation):
    out = A @ (d[:,None] * (B @ h_a) + h_b)

Distribution: 1D row-parallel over 8 NeuronCores; 512 rows per core.
Device does: projections (fp32 matmul), SpMM as dense bf16 matmuls with
exact small-int adjacency entries, two AllGathers (h_a, v).

The average degree is 32 and D=128, so a dense bf16 adjacency row (4096
* 2B = 8KB) costs the same HBM traffic as gathering ~32 sparse source
rows (32 * 128 * 2B = 8KB) — dense SpMM on the TensorEngine is the
right call at this density.

`reps` builds a NEFF that executes the whole computation N times
(statically unrolled; every rep re-reads inputs from DRAM and rewrites
the output). test.py uses two rep counts to measure per-iteration HW
time as a slope, cancelling per-call dispatch overhead.
"""

import numpy as np
import ml_dtypes

NA = 4096
NB = 4096
FA = 512
FB = 512
D = 128
M = 8  # cores
S = NA // M  # 512 rows per core
KT = NA // 128  # 32 k-tiles over the node dim
P = 128

_BUILT = {}


def _emit(nc, tc, tensors, reps=1, ablate=frozenset()):
    # ablate is a timing-attribution tool for test.py experiments only
    # ("coll" drops the collectives, "spmm" the big matmuls, "adj" the
    # adjacency DMA); results are wrong under any ablation.
    import concourse.mybir as mybir
    from concourse.masks import make_identity

    f32 = mybir.dt.float32
    bf16 = mybir.dt.bfloat16
    f8 = mybir.dt.float8e4
    f8e3 = mybir.dt.float8e3
    Relu = mybir.ActivationFunctionType.Relu
    Copy = mybir.ActivationFunctionType.Copy

    (xaT, xbT, Wa, Wb, ba_rep, bb_rep, ATs, BTs, dsw, outT) = tensors

    ctxs = []

    def pool(name, bufs, space="SBUF"):
        p = tc.tile_pool(name=name, bufs=bufs, space=space)
        ctxs.append(p)
        return p.__enter__()

    constp = pool("const", 1)
    bigp = pool("big", 1)
    workp = pool("work", 3)
    hlp = pool("hl", 4)
    psp = pool("ps", 2, "PSUM")
    accp = pool("acc", 2, "PSUM")
    trp = pool("tr", 2, "PSUM")
    dramp = pool("dram", 1, "DRAM")

    try:
        ident = constp.tile([P, P], f32, tag="ident")
        make_identity(nc, ident[:])

        # Software pipelining with a 2-rep skew: SpMM2 of rep r-2 is emitted
        # inside rep r, between proj_b(r) and SpMM1(r). On the in-order
        # TensorEngine this hides AG1(r)'s latency under SpMM2(r-2), and
        # gives AG2(r-1) a FULL iteration to complete before SpMM2(r-1)
        # consumes it in rep r+1 (a 1-rep skew left ~5us of AG2 exposed).
        def emit_spmm2(v_sb, at_sb, adjk):
            ops = accp.tile([P, S], f32, tag="acc2")
            kt2 = 1 if "spmm" in ablate else KT
            for k in range(kt2):
                nc.tensor.matmul(
                    ops[:],
                    lhsT=v_sb[:, k, :],
                    rhs=at_sb[:, adjk(k), :],
                    start=(k == 0),
                    stop=(k == kt2 - 1),
                )
            o_sb = workp.tile([P, S], f32, tag="osb")
            nc.vector.tensor_copy(o_sb[:], ops[:])
            nc.sync.dma_start(out=outT[:], in_=o_sb[:])

        pending = []  # [(v_sb, at_sb, adjk)] of up to 2 previous reps

        for rep in range(reps):
            # ---- resident loads ---------------------------------------------
            # Priority: consts (gate bias preload), xa (gates AG1 critical
            # path), bt (needed by SpMM1 right after AG1), xb, at (needed
            # last, by SpMM2).
            wa_sb = constp.tile([P, 4, D], bf16, tag="wa")
            wb_sb = constp.tile([P, 4, D], bf16, tag="wb")
            ba_sb = constp.tile([P, P], f32, tag="ba")
            bb_sb = constp.tile([P, P], f32, tag="bb")
            d_sb = constp.tile([P, 4], f32, tag="d")
            # x/W stay bf16: fp8 here was measured at rel err 1.6e-2 — the
            # rounding does NOT average down over the contraction (h is a
            # random-sign sum, so per-term relative error carries through)
            xa_sb = bigp.tile([P, 4, S], bf16, tag="xa")
            xb_sb = bigp.tile([P, 4, S], bf16, tag="xb")
            # adjacency double-buffered: rep r+1's A/B DMA prefetches under
            # rep r's SpMMs instead of serializing behind them.
            # fp8e4 (exact for the small-int edge counts, max ~4 << 240)
            # halves the dominant DMA stream; the matmuls stream it as the
            # moving operand in NORMAL mode (bf16 lhsT x fp8 rhs) — the
            # DoubleRow path measured slower and is deliberately avoided.
            # at spans two iterations under the 2-rep skew -> bufs=3
            at_sb = bigp.tile([P, KT, S], f8, tag="at", bufs=3)
            bt_sb = bigp.tile([P, KT, S], f8, tag="bt", bufs=2)

            nc.sync.dma_start(out=ba_sb[:], in_=ba_rep[:])
            nc.sync.dma_start(out=wa_sb[:], in_=Wa[:])
            nc.sync.dma_start(out=d_sb[:], in_=dsw[:])
            nc.sync.dma_start(out=xa_sb[:], in_=xaT[:])
            nc.sync.dma_start(out=bb_sb[:], in_=bb_rep[:])
            nc.sync.dma_start(out=wb_sb[:], in_=Wb[:])
            if "adj" not in ablate:
                nc.sync.dma_start(out=bt_sb[:], in_=BTs[:])
            else:
                nc.sync.dma_start(out=bt_sb[:, 0:2, :], in_=BTs[:, 0:2, :])
            nc.sync.dma_start(out=xb_sb[:], in_=xbT[:])
            if "adj" not in ablate:
                nc.sync.dma_start(out=at_sb[:], in_=ATs[:])
            else:
                nc.sync.dma_start(out=at_sb[:, 0:2, :], in_=ATs[:, 0:2, :])
            adjk = (lambda k: 0) if "adj" in ablate else (lambda k: k)

            def proj(x_sb, w_sb, b_sb, out_dt, tag):
                """4 node-major tiles [128 nodes, D], relu'd, bias from
                PSUM preload (all matmuls accumulate, start=False)."""
                outs = []
                for ri in range(4):
                    ps = psp.tile([P, D], f32, tag="proj")
                    nc.scalar.activation(ps[:], b_sb[:, 0:D], Copy)
                    for k in range(4):
                        nc.tensor.matmul(
                            ps[:],
                            lhsT=x_sb[:, k, ri * P : (ri + 1) * P],
                            rhs=w_sb[:, k, :],
                            start=False,
                            stop=(k == 3),
                        )
                    hf = hlp.tile([P, D], out_dt, tag=f"{tag}{ri}")
                    nc.scalar.activation(hf[:], ps[:], Relu)
                    outs.append(hf)
                return outs

            # ---- h_a: project, allgather (fp8e4) ----------------------------
            # h_a's fp8 rounding is doubly attenuated downstream (t=B@h_a
            # averages ~32 terms; d*t is ~1/3 of v): measured rel err goes
            # 1.29e-3 -> 1.9e-3, still 10x under the gate, while AG1 and the
            # gathered readback halve. SpMM1 runs fp8 x fp8 in NORMAL mode.
            ag_ha_in = dramp.tile([S, D], f8, tag=f"aghi{rep}")
            ag_ha_out = dramp.tile(
                [NA, D], f8, tag=f"agho{rep}", addr_space="Shared"
            )

            ha_tiles = proj(xa_sb, wa_sb, ba_sb, f8, "ha")
            for ri in range(4):
                nc.sync.dma_start(
                    out=ag_ha_in[ri * P : (ri + 1) * P, :], in_=ha_tiles[ri][:]
                )
            if "coll" not in ablate:
                nc.gpsimd.collective_compute(
                    "AllGather",
                    mybir.AluOpType.bypass,
                    replica_groups=[list(range(M))],
                    ins=[ag_ha_in[:].opt()],
                    outs=[ag_ha_out[:].opt()],
                )
            ha_sb = bigp.tile([P, KT, D], f8, tag="haf")
            _hav = ag_ha_out[:].rearrange("(k p) c -> p k c", p=P)
            for ci in range(8):
                ksl = slice(ci * KT // 8, (ci + 1) * KT // 8)
                nc.sync.dma_start(out=ha_sb[:, ksl, :], in_=_hav[:, ksl, :])

            # ---- h_b local (node-major fp32) --------------------------------
            hb_tiles = proj(xb_sb, wb_sb, bb_sb, f32, "hb")

            # ---- SpMM2 of rep r-2 (pipelined) -------------------------------
            if len(pending) == 2:
                emit_spmm2(*pending.pop(0))

            # ---- SpMM1: tT = (B_shard @ h_a)^T  [D, S] ----------------------
            tps = accp.tile([P, S], f32, tag="acc")
            kt1 = 1 if "spmm" in ablate else KT
            for k in range(kt1):
                nc.tensor.matmul(
                    tps[:],
                    lhsT=ha_sb[:, k, :],
                    rhs=bt_sb[:, adjk(k), :],
                    start=(k == 0),
                    stop=(k == kt1 - 1),
                )
            tT_sb = workp.tile([P, S], f32, tag="tT")
            nc.vector.tensor_copy(tT_sb[:], tps[:])

            # ---- v = d*t + h_b, node-major, fp8-e3m4, allgather -------------
            # e3m4 (4 mantissa bits): v's range fits +-15.5; values below
            # the 0.25 normal floor go subnormal but their ABSOLUTE error
            # (<=0.004) adds only ~4e-4 to the metric. Halves AG2 + the
            # gathered readback vs bf16.
            ag_v_in = dramp.tile([S, D], f8e3, tag=f"agvi{rep}")
            ag_v_out = dramp.tile(
                [NA, D], f8e3, tag=f"agvo{rep}", addr_space="Shared"
            )
            for ri in range(4):
                trps = trp.tile([P, P], f32, tag="tr")
                nc.tensor.transpose(
                    out=trps[:],
                    in_=tT_sb[:, ri * P : (ri + 1) * P],
                    identity=ident[:],
                )
                v32 = workp.tile([P, D], f32, tag="v32")
                nc.scalar.activation(
                    v32[:], trps[:], Copy, scale=d_sb[:, ri : ri + 1]
                )
                vl = hlp.tile([P, D], f8e3, tag=f"vl{ri}")
                nc.vector.tensor_add(out=vl[:], in0=v32[:], in1=hb_tiles[ri][:])
                nc.sync.dma_start(
                    out=ag_v_in[ri * P : (ri + 1) * P, :], in_=vl[:]
                )
            if "coll" not in ablate:
                nc.gpsimd.collective_compute(
                    "AllGather",
                    mybir.AluOpType.bypass,
                    replica_groups=[list(range(M))],
                    ins=[ag_v_in[:].opt()],
                    outs=[ag_v_out[:].opt()],
                )
            v_sb = bigp.tile([P, KT, D], f8e3, tag="vf", bufs=3)
            _vv = ag_v_out[:].rearrange("(k p) c -> p k c", p=P)
            for ci in range(8):
                ksl = slice(ci * KT // 8, (ci + 1) * KT // 8)
                nc.sync.dma_start(out=v_sb[:, ksl, :], in_=_vv[:, ksl, :])

            pending.append((v_sb, at_sb, adjk))

        # ---- drain: SpMM2 of the final rep(s) --------------------------------
        while pending:
            emit_spmm2(*pending.pop(0))
    finally:
        for p in reversed(ctxs):
            p.__exit__(None, None, None)


def _build(reps=1, ablate=frozenset()):
    key = ("nc", reps, tuple(sorted(ablate)))
    if key in _BUILT:
        return _BUILT[key]
    import concourse.bacc as bacc
    import concourse.mybir as mybir
    import concourse.tile as tile

    f32 = mybir.dt.float32
    bf16 = mybir.dt.bfloat16

    nc = bacc.Bacc("TRN2", target_bir_lowering=False, debug=False, num_devices=M)
    f8d = mybir.dt.float8e4
    xaT = nc.declare_dram_parameter("xaT", [P, 4, S], bf16, isOutput=False)
    xbT = nc.declare_dram_parameter("xbT", [P, 4, S], bf16, isOutput=False)
    Wa = nc.declare_dram_parameter("Wa", [P, 4, D], bf16, isOutput=False)
    Wb = nc.declare_dram_parameter("Wb", [P, 4, D], bf16, isOutput=False)
    ba = nc.declare_dram_parameter("ba", [P, P], f32, isOutput=False)
    bb = nc.declare_dram_parameter("bb", [P, P], f32, isOutput=False)
    ATs = nc.declare_dram_parameter("ATs", [P, KT, S], f8d, isOutput=False)
    BTs = nc.declare_dram_parameter("BTs", [P, KT, S], f8d, isOutput=False)
    dsw = nc.declare_dram_parameter("dsw", [P, 4], f32, isOutput=False)
    outT = nc.declare_dram_parameter("outT", [P, S], f32, isOutput=True)

    with tile.TileContext(nc) as tc:
        _emit(
            nc,
            tc,
            (
                xaT[:],
                xbT[:],
                Wa[:],
                Wb[:],
                ba[:],
                bb[:],
                ATs[:],
                BTs[:],
                dsw[:],
                outT[:],
            ),
            reps=reps,
            ablate=ablate,
        )
    nc.compile()
    _BUILT[key] = nc
    return nc


def _swz(a, kt):
    """[kt*128, n] row-major -> [128, kt, n] partition-major contiguous."""
    n = a.shape[1]
    return np.ascontiguousarray(a.reshape(kt, P, n).transpose(1, 0, 2))


def make_in_maps(x_a, x_b, W_a, b_a, W_b, b_b, edge_index_ab, edge_index_ba):
    bf = ml_dtypes.bfloat16
    f8 = ml_dtypes.float8_e4m3
    x_a = np.asarray(x_a, np.float32)
    x_b = np.asarray(x_b, np.float32)
    W_a = np.asarray(W_a, np.float32)
    W_b = np.asarray(W_b, np.float32)
    b_a = np.asarray(b_a, np.float32).reshape(-1)
    b_b = np.asarray(b_b, np.float32).reshape(-1)
    ea = np.asarray(edge_index_ab).astype(np.int64)
    eb = np.asarray(edge_index_ba).astype(np.int64)

    # Dense transposed adjacencies with duplicate accumulation.
    AT = (
        np.bincount(ea[1] * NA + ea[0], minlength=NA * NB)
        .reshape(NB, NA)
        .astype(np.float32)
    )  # AT[c, r] = A[r, c]
    BT = (
        np.bincount(eb[1] * NB + eb[0], minlength=NA * NB)
        .reshape(NA, NB)
        .astype(np.float32)
    )  # BT[c, r] = B[r, c]
    deg = (
        np.bincount(ea[1], minlength=NB) + np.bincount(eb[0], minlength=NB)
    ).astype(np.float32)
    d = np.where(
        deg > 0, np.float32(1.0) / np.maximum(deg, np.float32(1.0)), np.float32(0.0)
    ).astype(np.float32)

    xaT_f = np.ascontiguousarray(x_a.T).astype(bf)  # [FA, NA]
    xbT_f = np.ascontiguousarray(x_b.T).astype(bf)
    AT_bf = AT.astype(f8)  # exact: edge-count ints << 240
    BT_bf = BT.astype(f8)
    wa_sw = _swz(W_a, 4).astype(bf)
    wb_sw = _swz(W_b, 4).astype(bf)
    ba_rep = np.ascontiguousarray(np.broadcast_to(b_a, (P, P))).astype(np.float32)
    bb_rep = np.ascontiguousarray(np.broadcast_to(b_b, (P, P))).astype(np.float32)

    in_maps = []
    for m in range(M):
        sl = slice(m * S, (m + 1) * S)
        in_maps.append(
            {
                "xaT": _swz(xaT_f[:, sl], 4),
                "xbT": _swz(xbT_f[:, sl], 4),
                "Wa": wa_sw,
                "Wb": wb_sw,
                "ba": ba_rep,
                "bb": bb_rep,
                "ATs": _swz(np.ascontiguousarray(AT_bf[:, sl]), KT),
                "BTs": _swz(np.ascontiguousarray(BT_bf[:, sl]), KT),
                "dsw": np.ascontiguousarray(d[sl].reshape(4, P).T),
            }
        )
    return in_maps


def run(inputs, trace=False, reps=1):
    from concourse.bass_utils import run_bass_kernel_spmd

    nc = _build(reps=reps)
    in_maps = make_in_maps(**inputs)
    res = run_bass_kernel_spmd(nc, in_maps, core_ids=list(range(M)), trace=trace)
    out = np.concatenate([np.asarray(r["outT"]).T for r in res.results], axis=0)
    return out.astype(np.float32), res


def kernel(**inputs):
    # Plain bf16 for both gathered tensors: h_a's rounding error is
    # attenuated ~deg x by the d-normalization, v's averages down by
    # ~sqrt(deg) in A @ v. HW-measured rel err 1.29e-3 << 2e-2 gate.
    out, _ = run(inputs, trace=False)
    return out
